# revision 5
# baseline (speedup 1.0000x reference)
"""BinaryLinear kernel for Trainium2 (8 NeuronCores, SPMD).

Computes  out = sign(x) @ sign(W)^T * alpha  for
x: [8192, 2048] f32, W: [2048, 2048] f32, alpha: [1] f32.

Strategy: data-parallel over tokens (8 shards of 1024); the weight is
split 8 ways over out_features for distribution. Every core reads
slices {0,1,2,3} of W^T as f32 locally (identical on all cores, so the
program stays SPMD-uniform) plus its OWN 256-col slice, which it signs
to fp8 and contributes to an HBM AllGather; slots {4..7} of the
AllGather output supply the remaining columns. This cuts per-core HBM
traffic from 32 MB to ~21 MB (vs. replicating all of W in f32).

Numerics: x is signed to +-0.5 in ONE DVE op ((x>0) - 0.5), W to +-1
via ACT sign; fp8(E4M3) holds both exactly, PSUM accumulates exact
half-integers |sum| <= 1024, and the drain scales by 2*alpha. Output is
written as f16 (integers up to 2048 are exact) and converted to f32 on
the host, halving output traffic.

Engine plan: ACT = W signs; DVE = x signs then PSUM drains; GpSimd
issues the (blocking) AllGather; PE runs 8-matmul DoubleRow units
(pass, m-tile) emitted in predicted-data-arrival order. Rings:
scalar = alpha + my-slice + x + outputs; sync = W f32 slices {0..3};
vector = fp8 bounce-out + AllGather slot loads.
"""

import numpy as np

import concourse.bass as bass
import concourse.tile as tile
from concourse import bacc, mybir
from concourse.bass_utils import run_bass_kernel_spmd

N_CORES = 8
NTOK = 8192
INF = 2048
OUTF = 2048
TPC = NTOK // N_CORES  # tokens per core (1024)
P = 128
KT = INF // P  # 16 contraction tiles
MT = TPC // P  # 8 token tiles per core
SL = OUTF // N_CORES  # 256 out_features per W slice
FD = 512  # matmul moving free dim (one PSUM bank)

F32 = mybir.dt.float32
F16 = mybir.dt.float16
FP8 = mybir.dt.float8e4

KC = 2  # k-tiles per w0123 f32 chunk (512 KB)
KSL = 4  # k-tiles per wsl f32 chunk (512 KB)

# pass -> (use local bwl?, col offset in rhs tile, col range in full output)
# pass0 = slices {0,1} (cols 0:512, local f32), pass3 = slices {2,3}
# (cols 512:1024, local f32), pass1 = AG slots {4,5} (cols 1024:1536),
# pass2 = AG slots {6,7} (cols 1536:2048).
PASS_LOCAL = {0: True, 3: True, 1: False, 2: False}
PASS_NOFF = {0: 0, 3: FD, 1: 0, 2: FD}

# Static PE emission order, sorted by predicted data readiness:
# pass0 gated on w01 f32 (~12us) + x m-arrival; pass3 on w23 (~22us);
# passes 1/2 on the AllGather (~35us).
UNIT_ORDER = (
    [(0, 0), (0, 1), (0, 2), (0, 3), (0, 4), (0, 5), (0, 6)]
    + [(3, 0), (3, 1), (3, 2), (0, 7), (3, 3), (3, 4), (3, 5), (3, 6), (3, 7)]
    + [(1, m) for m in range(MT)]
    + [(2, m) for m in range(MT)]
)

_compiled = None
LAST_RESULT = None  # BassKernelResults of the most recent run (for profiling)


def _build():
    nc = bacc.Bacc(
        "TRN2",
        target_bir_lowering=False,
        debug=False,
        num_devices=N_CORES,
    )
    xt = nc.dram_tensor("xt", [MT * P * KT * P], F32, kind="ExternalInput").ap()
    wt = nc.dram_tensor("wt", [2 * KT * P * FD], F32, kind="ExternalInput").ap()
    wsl = nc.dram_tensor("wsl", [P * KT * SL], F32, kind="ExternalInput").ap()
    al = nc.dram_tensor("alpha", [P, 1], F32, kind="ExternalInput").ap()
    wsg_in = nc.dram_tensor("wsg_in", [P * KT * SL], FP8, kind="Internal")
    wsg_out = nc.dram_tensor(
        "wsg_out", [N_CORES * P * KT * SL], FP8, kind="Internal", addr_space="Shared"
    )
    out = nc.dram_tensor("out", [4, MT, P, FD], F16, kind="ExternalOutput").ap()

    with tile.TileContext(nc) as tc:
        with (
            tc.tile_pool(name="res", bufs=1) as res,
            tc.tile_pool(name="wload", bufs=3) as wload,
            tc.tile_pool(name="wsload", bufs=2) as wsload,
            tc.tile_pool(name="xload", bufs=3) as xload,
            tc.tile_pool(name="psum", bufs=8, space="PSUM") as ppool,
            tc.tile_pool(name="outp", bufs=4) as outp,
        ):
            bx = res.tile([P, KT, TPC], FP8)  # +-0.5 of x shard, 16 KB/part
            bwl = res.tile([P, KT, 2 * FD], FP8)  # slices 0-3, 16 KB/part
            bwr = res.tile([P, KT, 2 * FD], FP8)  # AG slots 4-7, 16 KB/part
            bsl = res.tile([P, KT, SL], FP8)  # my slice fp8, 4 KB/part
            alpha_t = res.tile([P, 1], F32)  # host-provided 2*alpha

            nc.scalar.dma_start(alpha_t[:], al)

            # -- my W slice: scalar ring f32 load -> ACT sign -> fp8 --
            for i in range(KT // KSL):
                wsf = wsload.tile([P, KSL, SL], F32, name="wsf", tag="wsf")
                src = wsl[i * P * KSL * SL : (i + 1) * P * KSL * SL].rearrange(
                    "(p f) -> p f", p=P
                )
                nc.scalar.dma_start(wsf[:].rearrange("p a b -> p (a b)"), src)
                nc.scalar.sign(bsl[:, i * KSL : (i + 1) * KSL, :], wsf[:])

            # -- bounce fp8 slice to HBM (gpsimd ring), then AllGather --
            nc.gpsimd.dma_start(
                wsg_in.ap().rearrange("(p f) -> p f", p=P),
                bsl[:].rearrange("p a b -> p (a b)"),
            )
            nc.gpsimd.collective_compute(
                "AllGather",
                mybir.AluOpType.bypass,
                replica_groups=[list(range(N_CORES))],
                ins=[wsg_in.ap()],
                outs=[wsg_out.ap()],
            )
            # AG slot loads (gpsimd ring, naturally after the AG): slots 4..7
            SLB = P * KT * SL
            for s in range(4, 8):
                src = wsg_out.ap()[s * SLB : (s + 1) * SLB].rearrange(
                    "(p f) -> p f", p=P
                )
                nc.gpsimd.dma_start(bwr[:, :, (s - 4) * SL : (s - 3) * SL], src)

            # -- w0123 f32 (sync ring) -> ACT sign -> bwl --
            off = 0
            for pair in range(2):
                for kc in range(KT // KC):
                    wf = wload.tile([P, KC, FD], F32, name="wf", tag="wf")
                    src = wt[off : off + P * KC * FD].rearrange("(p f) -> p f", p=P)
                    nc.sync.dma_start(wf[:].rearrange("p a b -> p (a b)"), src)
                    nc.scalar.sign(
                        bwl[:, kc * KC : (kc + 1) * KC, pair * FD : (pair + 1) * FD],
                        wf[:],
                    )
                    off += P * KC * FD

            # -- x m-chunks (scalar ring) -> DVE one-op sign to +-0.5 --
            for m in range(MT):
                xf = xload.tile([P, KT, P], F32, name="xf", tag="xf")
                src = xt[m * P * KT * P : (m + 1) * P * KT * P].rearrange(
                    "(p f) -> p f", p=P
                )
                nc.scalar.dma_start(xf[:].rearrange("p a b -> p (a b)"), src)
                nc.vector.tensor_scalar(
                    bx[:, :, m * P : (m + 1) * P], xf[:], 0.0, 0.5,
                    op0=mybir.AluOpType.is_gt, op1=mybir.AluOpType.subtract,
                )

            # -- PE units: 8 DoubleRow matmuls + DVE drain + out DMA --
            for p, m in UNIT_ORDER:
                rhs_t = bwl if PASS_LOCAL[p] else bwr
                noff = PASS_NOFF[p]
                ps = ppool.tile([P, FD], F32, name="ps", tag="ps")
                for kc in range(KT // 2):
                    nc.tensor.matmul(
                        ps[:],
                        bx[:, 2 * kc : 2 * kc + 2, m * P : (m + 1) * P],
                        rhs_t[:, 2 * kc : 2 * kc + 2, noff : noff + FD],
                        start=(kc == 0),
                        stop=(kc == KT // 2 - 1),
                        perf_mode=mybir.MatmulPerfMode.DoubleRow,
                    )
                ob = outp.tile([P, FD], F16, name="ob", tag="ob")
                nc.vector.tensor_scalar_mul(ob[:], ps[:], alpha_t[:])
                nc.scalar.dma_start(out[p, m], ob[:])

    nc.compile()
    return nc


def _pack_common(weight):
    WT4 = np.ascontiguousarray(weight.T).reshape(KT, P, OUTF)
    parts = []
    for pair in range(2):
        cols = slice(pair * FD, (pair + 1) * FD)
        for kc in range(KT // KC):
            parts.append(WT4[kc * KC : (kc + 1) * KC, :, cols].transpose(1, 0, 2).ravel())
    wt_flat = np.ascontiguousarray(np.concatenate(parts))
    wsls = []
    for c in range(N_CORES):
        cols = slice(c * SL, (c + 1) * SL)
        ps = []
        for i in range(KT // KSL):
            ps.append(
                WT4[i * KSL : (i + 1) * KSL, :, cols].transpose(1, 0, 2).ravel()
            )
        wsls.append(np.ascontiguousarray(np.concatenate(ps)))
    return wt_flat, wsls


def _pack_x_shard(xs):
    xT4 = np.ascontiguousarray(xs.T).reshape(KT, P, TPC)
    return np.ascontiguousarray(
        np.concatenate(
            [xT4[:, :, m * P : (m + 1) * P].transpose(1, 0, 2).ravel() for m in range(MT)]
        )
    )


def kernel(x, weight, alpha):
    global _compiled, LAST_RESULT
    if _compiled is None:
        _compiled = _build()
    nc = _compiled

    x = np.asarray(x, dtype=np.float32)
    weight = np.asarray(weight, dtype=np.float32)
    alpha = np.asarray(alpha, dtype=np.float32)

    wt_flat, wsls = _pack_common(weight)
    alv = np.full((P, 1), 2.0 * float(alpha.reshape(-1)[0]), dtype=np.float32)
    in_maps = []
    for c in range(N_CORES):
        xs = _pack_x_shard(x[c * TPC : (c + 1) * TPC, :])
        in_maps.append({"xt": xs, "wt": wt_flat, "wsl": wsls[c], "alpha": alv})

    LAST_RESULT = run_bass_kernel_spmd(nc, in_maps, list(range(N_CORES)))
    full = np.empty((NTOK, OUTF), dtype=np.float32)
    # pass -> full-output column offset
    pass_cols = {0: 0, 3: FD, 1: 2 * FD, 2: 3 * FD}
    for c in range(N_CORES):
        o = LAST_RESULT.results[c]["out"]  # [4, MT, P, FD] f16
        blk = o.astype(np.float32).reshape(4, TPC, FD)
        for p in range(4):
            full[c * TPC : (c + 1) * TPC, pass_cols[p] : pass_cols[p] + FD] = blk[p]
    return full
